# revision 80
# baseline (speedup 1.0000x reference)
"""Trainium2 Bass kernel for MultiHeadAttention (B=4, S=1024, E=1024, H=16, Dh=64).

Sharding: 8 cores = (batch b in 0..3) x (head-group hg in 0..1, 8 heads each).
The reference reshapes [B,H,S,Dh] -> [B,S,E] WITHOUT transposing heads back, so
head h's attention output occupies output rows t' = h*64 + s//16 and the final
projection is row-parallel across head groups: no cross-core communication.

Fast path (causal mask, zero biases):
  Q/K/V projections run as 3-term error-compensated fp8e4m3 DoubleRow matmuls
  (x and w split hi+lo on host at shared power-of-2 scales; the dropped lo@lo
  term is ~eps^2 ~ 0.1%).  PSUM stays scaled; descales fold into the exp scale
  (Q,K) and the ones-columns of V' (V).  Causal masking is a window-only
  (128-col) affine_select on the idle GPSIMD engine after exp.  q/k, exp
  output, V, the output projection and the output DMA are bf16; z stays in
  fp32 PSUM.  Normalization (1/colsums) is fused into the scrambled gather.
  Schedule: Q/K for dt0/dt1 run chunk-major against the input DMA across 8
  PSUM accumulators; dt2/dt3 Q/K, V slices, z and O slices interleave with
  the per-head-pair scores stream at tile granularity so PE stays busy while
  ACT runs exp.  End-to-end rel err ~ 1.0e-2 (threshold 2e-2;
  deterministic inputs make this exact).
"""
import numpy as np

B, S, E, H, DH = 4, 1024, 1024, 16, 64
NCORES = 8
HPC = 8          # heads per core
EC = 8           # 128-row chunks of E
TT = 8           # 128-row t-tiles of S
NJ = 2           # 512-col s-blocks

SX = 16.0        # fp8 scale for x
SW = 512.0       # fp8 scale for weights
SV = SX * SW     # v/ones scale (2**13)
SE = 0.5 / (SX * SW) ** 2   # exp scale absorbing q/k descale (2**-27)

_CACHE = {}


def _build_fast():
    import concourse.bacc as bacc
    import concourse.tile as tile
    import concourse.mybir as mybir

    f32 = mybir.dt.float32
    f32r = mybir.dt.float32r
    bf16 = mybir.dt.bfloat16
    f8 = mybir.dt.float8e4
    DR = mybir.MatmulPerfMode.DoubleRow
    Exp = mybir.ActivationFunctionType.Exp
    mult = mybir.AluOpType.mult
    is_ge = mybir.AluOpType.is_ge

    nc = bacc.Bacc("TRN2")
    xh = nc.dram_tensor("xh", [128, EC, S], f8, kind="ExternalInput")
    xl = nc.dram_tensor("xl", [128, EC, S], f8, kind="ExternalInput")
    wqh = nc.dram_tensor("wqh", [128, EC, 512], f8, kind="ExternalInput")
    wql = nc.dram_tensor("wql", [128, EC, 512], f8, kind="ExternalInput")
    wkh = nc.dram_tensor("wkh", [128, EC, 512], f8, kind="ExternalInput")
    wkl = nc.dram_tensor("wkl", [128, EC, 512], f8, kind="ExternalInput")
    wvh = nc.dram_tensor("wvh", [128, EC, 512], f8, kind="ExternalInput")
    wvl = nc.dram_tensor("wvl", [128, EC, 512], f8, kind="ExternalInput")
    wo16 = nc.dram_tensor("wo16", [128, EC, 1024], bf16, kind="ExternalInput")
    out = nc.dram_tensor("out", [4, 2, 128, 512], bf16, kind="ExternalOutput")

    with tile.TileContext(nc) as tc:
        with (
            tc.tile_pool(name="pp", bufs=1) as pp,
            tc.tile_pool(name="expa", bufs=16) as expa,
            tc.tile_pool(name="expb", bufs=8) as expb,
            tc.tile_pool(name="sml", bufs=4) as sml,
            tc.tile_pool(name="outp", bufs=2) as outp,
        ):
            xh_sb = pp.tile([128, EC, S], f8)
            xl_sb = pp.tile([128, EC, S], f8)
            wqh_sb = pp.tile([128, EC, 512], f8)
            wql_sb = pp.tile([128, EC, 512], f8)
            wkh_sb = pp.tile([128, EC, 512], f8)
            wkl_sb = pp.tile([128, EC, 512], f8)
            wvh_sb = pp.tile([128, EC, 512], f8)
            wvl_sb = pp.tile([128, EC, 512], f8)
            wo_sb = pp.tile([128, EC, 1024], bf16)
            qt_sb = pp.tile([128, 4, S], bf16)
            kt_sb = pp.tile([128, 4, S], bf16)
            vp_sb = pp.tile([128, TT, 1024], bf16)
            x2t_sb = pp.tile([128, EC, 512], bf16)

            # Half-tensor DMAs in consumption order: pair-chunked DMAs paid
            # too much per-op HWDGE/dge overhead, full-tensor ones delayed the
            # first matmul too long - 4-chunk halves are the sweet spot.
            for c in range(0, EC, 4):
                nc.sync.dma_start(out=xh_sb[:, c:c + 4], in_=xh[:, c:c + 4])
                nc.sync.dma_start(out=wqh_sb[:, c:c + 4], in_=wqh[:, c:c + 4])
                nc.sync.dma_start(out=wql_sb[:, c:c + 4], in_=wql[:, c:c + 4])
                nc.sync.dma_start(out=xl_sb[:, c:c + 4], in_=xl[:, c:c + 4])
            for c in range(0, EC, 4):
                nc.sync.dma_start(out=wkh_sb[:, c:c + 4], in_=wkh[:, c:c + 4])
                nc.sync.dma_start(out=wkl_sb[:, c:c + 4], in_=wkl[:, c:c + 4])
            nc.sync.dma_start(out=wvh_sb, in_=wvh.ap())
            nc.sync.dma_start(out=wvl_sb, in_=wvl.ap())
            for c in range(0, EC, 4):
                nc.sync.dma_start(out=wo_sb[:, c:c + 4], in_=wo16[:, c:c + 4])

            # ones columns of V' scaled by SV (descales z and sums together)
            vview = vp_sb.rearrange("p t (h two d) -> p t h two d", two=2, d=DH)
            for tt in range(TT):
                nc.gpsimd.memset(vview[:, tt, :, 1, :], float(SV))

            # ---- Q^T / K^T projections: fp8 DR 3-term, PSUM stays scaled.
            # Phase A covers only dt0/dt1 (heads of pairs 0-1), chunk-major
            # over 8 concurrent PSUM accumulators so PE streams against the
            # input DMA.  dt2/dt3 run later as attention fillers (q_fill).
            # smp is allocated before phase A; K dt1's two accumulators live
            # in smp slots so qkp (Q + K dt0) can release - and mm allocate -
            # after only those six copies.
            smp = tc.alloc_tile_pool(name="sm", bufs=2, space="PSUM")
            qkp = tc.alloc_tile_pool(name="qkp", bufs=1, space="PSUM")
            qps = [qkp.tile([128, 512], f32, name=f"qk_{g}") for g in range(6)]
            qps += [smp.tile([128, 512], f32, tag="sm", name=f"qk_{g}")
                    for g in (6, 7)]
            def qk_copy(pi, dt_, sh, dest):
                g = 4 * pi + 2 * dt_ + sh
                if (2 * dt_ + sh) % 2 == 0:
                    nc.scalar.copy(
                        out=dest[:, dt_, 512 * sh:512 * sh + 512], in_=qps[g])
                else:
                    nc.vector.tensor_copy(
                        dest[:, dt_, 512 * sh:512 * sh + 512], qps[g])
            for pi, (wh_sb, wl_sb, dest) in enumerate((
                (wqh_sb, wql_sb, qt_sb),
                (wkh_sb, wkl_sb, kt_sb),
            )):
                for ci, c in enumerate(range(0, EC, 2)):
                    for dt_ in range(2):
                        for sh in range(2):
                            for ti, (a_sb, w_sb) in enumerate((
                                (xh_sb, wh_sb), (xh_sb, wl_sb), (xl_sb, wh_sb)
                            )):
                                nc.tensor.matmul(
                                    qps[4 * pi + 2 * dt_ + sh],
                                    w_sb[:, c:c + 2, 128 * dt_:128 * dt_ + 128],
                                    a_sb[:, c:c + 2, 512 * sh:512 * sh + 512],
                                    start=(ci == 0 and ti == 0),
                                    stop=(ci == 3 and ti == 2),
                                    perf_mode=DR,
                                )
                if pi == 0:
                    for dt_ in range(2):
                        for sh in range(2):
                            qk_copy(0, dt_, sh, dest)
            for sh in range(2):
                qk_copy(1, 0, sh, kt_sb)
            for sh in range(2):
                qk_copy(1, 1, sh, kt_sb)
            qkp.release()
            mm = tc.alloc_tile_pool(name="mm", bufs=3, space="PSUM")

            def qk_fill(pi, dt_, sh):
                # one (proj, dt, sh) projection group for dt2/dt3, run as an
                # attention filler from the small PSUM pool (inputs resident)
                wh_sb, wl_sb, dest = (
                    (wqh_sb, wql_sb, qt_sb), (wkh_sb, wkl_sb, kt_sb))[pi]
                ps = smp.tile([128, 512], f32, tag="sm",
                              name=f"qkf_{pi}_{dt_}_{sh}")
                k = 0
                for a_sb, w_sb in (
                    (xh_sb, wh_sb), (xh_sb, wl_sb), (xl_sb, wh_sb)
                ):
                    for c in range(0, EC, 2):
                        nc.tensor.matmul(
                            ps,
                            w_sb[:, c:c + 2, 128 * dt_:128 * dt_ + 128],
                            a_sb[:, c:c + 2, 512 * sh:512 * sh + 512],
                            start=(k == 0), stop=(k == 11), perf_mode=DR,
                        )
                        k += 1
                nc.vector.tensor_copy(
                    dest[:, dt_, 512 * sh:512 * sh + 512], ps)

            # ---- per head-pair: V proj, scores+exp, z+normalize, O proj,
            # interleaved at tile granularity so PE keeps working while ACT
            # runs exp (exp throughput < PE scores throughput). ----
            def v_piece(hp, g):
                # V projection for tt group g (4 t-tiles), both heads of pair
                vps = smp.tile([128, 512], f32, tag="sm", name=f"vps_{hp}_{g}")
                for ti in range(4):
                    tt = 4 * g + ti
                    k = 0
                    for a_sb, w_sb in (
                        (xh_sb, wvh_sb), (xh_sb, wvl_sb), (xl_sb, wvh_sb)
                    ):
                        for c in range(0, EC, 2):
                            nc.tensor.matmul(
                                vps[:, 128 * ti:128 * ti + 128],
                                a_sb[:, c:c + 2, 128 * tt:128 * tt + 128],
                                w_sb[:, c:c + 2, 128 * hp:128 * hp + 128],
                                start=(k == 0), stop=(k == 11),
                                perf_mode=DR,
                            )
                            k += 1
                src = vps.rearrange("p (t h d) -> p t h d", t=4, h=2, d=DH)
                nc.vector.tensor_copy(
                    vview[:, 4 * g:4 * g + 4, 2 * hp:2 * hp + 2, 0, :], src
                )

            def scores_piece(hp, tt, et):
                c0 = 128 * tt
                js = [j for j in range(NJ) if c0 <= 512 * j + 511]
                s0 = 512 * js[0]
                width = 1024 - s0
                heads = (2 * hp, 2 * hp + 1)
                if width == 512:
                    # both heads share one [128,1024] psum tile + one exp
                    ps = mm.tile([128, 1024], f32, tag="mm")
                    e = expb.tile([128, 1024], bf16, tag="expb",
                                  name=f"e_{heads[0]}_{tt}")
                    for hi, h in enumerate(heads):
                        dt_, pb, ho = h // 2, 64 * (h % 2), 512 * hi
                        lo = max(512, c0)
                        nc.tensor.matmul(
                            ps[:, ho + lo - s0:ho + 512],
                            kt_sb[pb:pb + 64, dt_, c0:c0 + 128],
                            qt_sb[pb:pb + 64, dt_, lo:1024],
                            start=True, stop=True,
                        )
                    pv = ps.rearrange("p (two s) -> p two s", two=2)
                    ev = e.rearrange("p (two s) -> p two s", two=2)
                    nc.scalar.activation(
                        ev[:, :, c0 - s0:], pv[:, :, c0 - s0:], Exp, scale=SE
                    )
                    for hi, h in enumerate(heads):
                        w0 = 512 * hi + c0 - s0
                        nc.gpsimd.affine_select(
                            out=e[:, w0:w0 + 128], in_=e[:, w0:w0 + 128],
                            pattern=[[1, 128]], compare_op=is_ge,
                            fill=0.0, base=0, channel_multiplier=-1,
                        )
                        et[(h, tt, 1)] = e[:, 512 * hi:512 * hi + 512]
                else:
                    for h in heads:
                        dt_, pb = h // 2, 64 * (h % 2)
                        ps = mm.tile([128, width], f32, tag="mm")
                        for j in js:
                            lo = max(512 * j, c0)
                            nc.tensor.matmul(
                                ps[:, lo - s0:512 * j + 512 - s0],
                                kt_sb[pb:pb + 64, dt_, c0:c0 + 128],
                                qt_sb[pb:pb + 64, dt_, lo:512 * j + 512],
                                start=True, stop=True,
                            )
                        e = expa.tile([128, width], bf16, tag="expa",
                                      name=f"e_{h}_{tt}")
                        nc.scalar.activation(
                            e[:, c0 - s0:], ps[:, c0 - s0:], Exp, scale=SE
                        )
                        nc.gpsimd.affine_select(
                            out=e[:, c0 - s0:c0 - s0 + 128],
                            in_=e[:, c0 - s0:c0 - s0 + 128],
                            pattern=[[1, 128]], compare_op=is_ge,
                            fill=0.0, base=0, channel_multiplier=-1,
                        )
                        for j in js:
                            et[(h, tt, j)] = e[:, 512 * j - s0:512 * j - s0 + 512]

            def z_piece(hp, h, j, et, recs, pool=None):
                if h not in recs:
                    recs[h] = sml.tile([64, S], f32, tag="rec", name=f"rec_{h}")
                rec = recs[h]
                pool = pool or smp
                zt = pool.tile([128, 512], f32, tag=pool.name, name=f"zt_{h}_{j}")
                ks = [tt for tt in range(TT) if (h, tt, j) in et]
                for i, tt in enumerate(ks):
                    lo = max(0, 128 * tt - 512 * j)
                    nc.tensor.matmul(
                        zt[:, lo:], vp_sb[:, tt, 128 * h:128 * h + 128],
                        et[(h, tt, j)][:, lo:],
                        start=(i == 0), stop=(i == len(ks) - 1),
                    )
                zv = zt[0:64].rearrange("p (m c par) -> par p c m", m=32, c=8, par=2)
                sv_ = zt[64:128].rearrange("p (m c par) -> par p c m", m=32, c=8, par=2)
                rv = rec[:, 512 * j:512 * j + 512].rearrange(
                    "p (m c par) -> par p c m", m=32, c=8, par=2)
                if hp == 3 and h == 7 and j == 1:
                    # terminal chain: split by ec-halves so the O projection's
                    # early k-chunks unblock after the first half
                    for cl, ch in ((0, 4), (4, 8)):
                        nc.vector.reciprocal(rv[0][:, cl:ch, :], sv_[0][:, cl:ch, :])
                        nc.vector.reciprocal(rv[1][:, cl:ch, :], sv_[1][:, cl:ch, :])
                        for P in range(2):
                            nc.vector.tensor_tensor(
                                x2t_sb[64 * P:64 * P + 64, cl:ch,
                                       64 * h + 32 * j:64 * h + 32 * j + 32],
                                zv[P][:, cl:ch, :], rv[P][:, cl:ch, :], op=mult,
                            )
                else:
                    nc.vector.reciprocal(rec[:, 512 * j:512 * j + 512], zt[64:128, :])
                    for P in range(2):
                        nc.vector.tensor_tensor(
                            x2t_sb[64 * P:64 * P + 64, :,
                                   64 * h + 32 * j:64 * h + 32 * j + 32],
                            zv[P], rv[P], op=mult,
                        )

            def o_piece(hp, eh, osbs):
                if hp not in osbs:
                    osbs[hp] = outp.tile([128, 1024], bf16, tag="osb", name=f"osb_{hp}")
                osb = osbs[hp]
                pool = mm if hp == 3 else smp
                ps = pool.tile([128, 512], f32, tag=pool.name,
                               name=f"ops_{hp}_{eh}")
                for c in range(EC):
                    nc.tensor.matmul(
                        ps, x2t_sb[:, c, 128 * hp:128 * hp + 128],
                        wo_sb[:, c, 512 * eh:512 * eh + 512],
                        start=(c == 0), stop=(c == EC - 1),
                    )
                if eh == 0 and hp == 3:
                    # last pair: ACT is idle, DVE still owns the norm chain
                    nc.scalar.copy(out=osb[:, 0:512], in_=ps)
                else:
                    nc.vector.tensor_copy(osb[:, 512 * eh:512 * eh + 512], ps)
                nc.sync.dma_start(
                    out=out[hp, eh],
                    in_=osb[:, 512 * eh:512 * eh + 512],
                )

            ets = {}
            recs = {}
            osbs = {}
            carry = []
            for p in range(5):
                # early fillers: finish pair p-1 (z j=1, O), start V of pair p;
                # late fillers: z j=0 of pair p (needs exp of tt 0-3 only).
                fill = []
                if p >= 1:
                    q = p - 1
                    zp = mm if p == 4 else None  # drain: mm slots are free
                    zj1 = [
                        lambda h=h, q=q, zp=zp: z_piece(
                            q, 2 * q + h, 1, ets[q], recs, zp)
                        for h in range(2)
                    ]
                    if p == 4:
                        # drain: interleave pair-2's deferred O slices between
                        # the z groups to cover exp-backlog and DVE-norm waits
                        fill += [zj1[0], carry.pop(0), zj1[1]] + carry
                        carry = []
                    else:
                        fill += zj1
                if p < 4:
                    fill += [lambda g=g, p=p: v_piece(p, g) for g in range(2)]
                if p >= 1:
                    q = p - 1
                    fill += [lambda eh=eh, q=q: o_piece(q, eh, osbs)
                             for eh in range(2)]
                if p == 0:
                    # project dt2 fully and Q-dt3 during pair 0 (8 fillers for
                    # 8 slots); K-dt3 lands in pair 1, balancing its load
                    fill += [
                        lambda pi=pi, dt_=dt_, sh=sh: qk_fill(pi, dt_, sh)
                        for pi, dt_ in ((0, 2), (1, 2), (0, 3))
                        for sh in range(2)
                    ]
                elif p == 1:
                    fill += [lambda sh=sh: qk_fill(1, 3, sh) for sh in range(2)]
                late = []
                if p < 4:
                    lp = mm if p == 3 else None
                    late = [
                        lambda h=h, p=p, lp=lp: z_piece(
                            p, 2 * p + h, 0, ets.setdefault(p, {}), recs, lp)
                        for h in range(2)
                    ]
                if p == 3:
                    # front-load the last pair's scores 2:1 so ACT starts its
                    # final exp burst earlier; O(2) pieces carry to the drain
                    ets.setdefault(p, {})
                    zs = fill[:2] + fill[4:]      # z(2,*,1), then O(2,*)
                    vs = fill[2:4]                # V(3) pieces
                    for tt in range(TT):
                        scores_piece(p, tt, ets[p])
                        if tt in (1, 3) and zs:
                            zs.pop(0)()
                        elif tt in (4, 5) and vs:
                            vs.pop(0)()
                        elif tt >= 6 and late:
                            late.pop(0)()
                    fill = zs
                elif p < 4:
                    ets.setdefault(p, {})
                    for tt in range(TT):
                        scores_piece(p, tt, ets[p])
                        if tt >= 6 and late:
                            late.pop(0)()
                        elif fill:
                            fill.pop(0)()
                if p == 3:
                    carry = fill
                    fill = []
                for f in fill + late:
                    f()
            mm.release()
            smp.release()
    nc.compile()
    return nc


def _build_slow(variant):
    """Baseline fp32r kernel — fallback for nonzero q/k biases or odd masks."""
    import concourse.bacc as bacc
    import concourse.tile as tile
    import concourse.mybir as mybir

    f32 = mybir.dt.float32
    f32r = mybir.dt.float32r
    Exp = mybir.ActivationFunctionType.Exp
    mult = mybir.AluOpType.mult
    is_ge = mybir.AluOpType.is_ge

    causal = variant == "slow_causal"

    def computed(tt, j):
        if not causal:
            return True
        return 128 * tt <= 512 * j + 511

    nc = bacc.Bacc("TRN2")
    xt = nc.dram_tensor("xt", [128, EC, S], f32r, kind="ExternalInput")
    wq = nc.dram_tensor("wq", [128, EC, 512], f32r, kind="ExternalInput")
    wk = nc.dram_tensor("wk", [128, EC, 512], f32r, kind="ExternalInput")
    wv = nc.dram_tensor("wv", [128, EC, 512], f32r, kind="ExternalInput")
    wo = nc.dram_tensor("wo", [128, EC, 1024], f32r, kind="ExternalInput")
    bq = nc.dram_tensor("bq", [128, 4], f32, kind="ExternalInput")
    bk = nc.dram_tensor("bk", [128, 4], f32, kind="ExternalInput")
    if not causal:
        mkt = nc.dram_tensor("mkt", [128, TT, S], f32, kind="ExternalInput")
    out = nc.dram_tensor("out", [4, 128, 1024], f32, kind="ExternalOutput")

    with tile.TileContext(nc) as tc:
        with (
            tc.tile_pool(name="persist", bufs=1) as pp,
            tc.tile_pool(name="mm", bufs=3, space="PSUM") as mm,
            tc.tile_pool(name="ztp", bufs=2, space="PSUM") as ztp,
        ):
            p1 = tc.alloc_tile_pool(name="p1", bufs=1)
            xt_sb = p1.tile([128, EC, S], f32r)
            wq_sb = p1.tile([128, EC, 512], f32r)
            wk_sb = p1.tile([128, EC, 512], f32r)
            wv_sb = p1.tile([128, EC, 512], f32r)
            for k in range(0, EC, 2):
                nc.sync.dma_start(out=xt_sb[:, k:k + 2, :], in_=xt[:, k:k + 2, :])
                nc.sync.dma_start(out=wq_sb[:, k:k + 2, :], in_=wq[:, k:k + 2, :])
                nc.sync.dma_start(out=wk_sb[:, k:k + 2, :], in_=wk[:, k:k + 2, :])
                nc.sync.dma_start(out=wv_sb[:, k:k + 2, :], in_=wv[:, k:k + 2, :])
            qt_sb = pp.tile([128, 4, S], bf16)
            kt_sb = pp.tile([128, 4, S], bf16)
            vp_sb = pp.tile([128, TT, 1024], f32r)
            x2t_sb = pp.tile([128, EC, 512], f32r)
            bq_sb = pp.tile([128, 4], f32)
            bk_sb = pp.tile([128, 4], f32)
            if not causal:
                mkt_sb = pp.tile([128, TT, S], f32)
                nc.sync.dma_start(out=mkt_sb, in_=mkt.ap())
            nc.sync.dma_start(out=bq_sb, in_=bq.ap())
            nc.sync.dma_start(out=bk_sb, in_=bk.ap())

            vview = vp_sb.rearrange("p t (h two d) -> p t h two d", two=2, d=DH)
            ones_sb = pp.tile([128, 512], f32)
            nc.vector.memset(ones_sb, 1.0)
            ones_v = ones_sb.rearrange("p (h d) -> p h d", d=DH)
            for tt in range(TT):
                nc.vector.tensor_copy(vview[:, tt, :, 1, :], ones_v)

            for wsb, dest, bias in ((wq_sb, qt_sb, bq_sb), (wk_sb, kt_sb, bk_sb)):
                for dt_ in range(4):
                    for sh in range(2):
                        ps = mm.tile([128, 512], f32, tag="mm")
                        for ec in range(EC):
                            nc.tensor.matmul(
                                ps, wsb[:, ec, 128 * dt_:128 * dt_ + 128],
                                xt_sb[:, ec, 512 * sh:512 * sh + 512],
                                start=(ec == 0), stop=(ec == EC - 1),
                            )
                        nc.vector.tensor_scalar_add(
                            out=dest[:, dt_, 512 * sh:512 * sh + 512],
                            in0=ps, scalar1=bias[:, dt_:dt_ + 1],
                        )
            for tt in range(TT):
                ps = mm.tile([128, 512], f32, tag="mm")
                for ec in range(EC):
                    nc.tensor.matmul(
                        ps, xt_sb[:, ec, 128 * tt:128 * tt + 128],
                        wv_sb[:, ec, :],
                        start=(ec == 0), stop=(ec == EC - 1),
                    )
                nc.vector.tensor_copy(
                    vview[:, tt, :, 0, :], ps.rearrange("p (h d) -> p h d", d=DH)
                )
            p1.release()
            late = tc.alloc_tile_pool(name="late", bufs=1)
            expa = tc.alloc_tile_pool(name="expa", bufs=8)
            expb = tc.alloc_tile_pool(name="expb", bufs=8)
            small = tc.alloc_tile_pool(name="small", bufs=2)
            outp = tc.alloc_tile_pool(name="outp", bufs=2)
            wo_sb = late.tile([128, EC, 1024], f32r)
            nc.sync.dma_start(out=wo_sb, in_=wo.ap())

            for hp in range(HPC // 2):
                pair = (2 * hp, 2 * hp + 1)
                et = {}
                for tt in range(TT):
                    js = [j for j in range(NJ) if computed(tt, j)]
                    s0 = 512 * js[0]
                    c0 = 128 * tt
                    pss = {}
                    for h in pair:
                        dt_ = h // 2
                        pb = 64 * (h % 2)
                        ps = mm.tile([128, 1024], f32, tag="mm", name=f"ps_{h}_{tt}")
                        pss[h] = ps
                        for j in js:
                            lo = max(512 * j, c0) if causal else 512 * j
                            nc.tensor.matmul(
                                ps[:, lo - s0:512 * j + 512 - s0],
                                kt_sb[pb:pb + 64, dt_, c0:c0 + 128],
                                qt_sb[pb:pb + 64, dt_, lo:512 * j + 512],
                                start=True, stop=True,
                            )
                            if not causal:
                                o = 512 * j - s0
                                nc.vector.tensor_add(
                                    ps[:, o:o + 512],
                                    ps[:, o:o + 512],
                                    mkt_sb[:, tt, 512 * j:512 * j + 512],
                                )
                    for h in pair:
                        ps = pss[h]
                        if tt < 4 or not causal:
                            e = expa.tile([128, 1024], f32r, tag="expa",
                                          name=f"e_{h}_{tt}")
                        else:
                            e = expb.tile([128, 512], f32r, tag="expb",
                                          name=f"e_{h}_{tt}")
                        if causal:
                            nc.scalar.activation(
                                e[:, c0 - s0:], ps[:, c0 - s0:1024 - s0],
                                Exp, scale=0.5,
                            )
                            nc.gpsimd.affine_select(
                                out=e[:, 0:c0 + 128 - s0], in_=e[:, 0:c0 + 128 - s0],
                                pattern=[[1, c0 + 128 - s0]], compare_op=is_ge,
                                fill=0.0, base=s0 - c0, channel_multiplier=-1,
                            )
                        else:
                            nc.scalar.activation(
                                e[:, :], ps[:, :1024 - s0], Exp, scale=0.5
                            )
                        for j in js:
                            et[(h, tt, j)] = e[:, 512 * j - s0:512 * j - s0 + 512]
                for h in pair:
                    zt_f = small.tile([64, S], f32, tag="ztf", name=f"ztf_{h}")
                    rec = small.tile([64, S], f32, tag="rec", name=f"rec_{h}")
                    for j in range(NJ):
                        zt = ztp.tile([128, 512], f32, tag="zt", name=f"zt_{h}_{j}")
                        ks = [tt for tt in range(TT) if (h, tt, j) in et]
                        for i, tt in enumerate(ks):
                            lo = max(0, 128 * tt - 512 * j) if causal else 0
                            nc.tensor.matmul(
                                zt[:, lo:], vp_sb[:, tt, 128 * h:128 * h + 128],
                                et[(h, tt, j)][:, lo:],
                                start=(i == 0), stop=(i == len(ks) - 1),
                            )
                        nc.vector.reciprocal(rec[:, 512 * j:512 * j + 512],
                                             zt[64:128, :])
                        nc.vector.tensor_copy(zt_f[:, 512 * j:512 * j + 512],
                                              zt[0:64, :])
                    zv = zt_f.rearrange("p (m c par) -> par p c m", m=64, c=8, par=2)
                    rv = rec.rearrange("p (m c par) -> par p c m", m=64, c=8, par=2)
                    for P in range(2):
                        nc.vector.tensor_tensor(
                            x2t_sb[64 * P:64 * P + 64, :, 64 * h:64 * h + 64],
                            zv[P], rv[P], op=mult,
                        )

            for tp in range(4):
                osb = outp.tile([128, 1024], f32, tag="osb")
                for eh in range(2):
                    ps = mm.tile([128, 512], f32, tag="mm")
                    for c in range(EC):
                        nc.tensor.matmul(
                            ps, x2t_sb[:, c, 128 * tp:128 * tp + 128],
                            wo_sb[:, c, 512 * eh:512 * eh + 512],
                            start=(c == 0), stop=(c == EC - 1),
                        )
                    nc.vector.tensor_copy(osb[:, 512 * eh:512 * eh + 512], ps)
                nc.sync.dma_start(out=out[tp], in_=osb)
            for p in (outp, small, expb, expa, late):
                p.release()
    nc.compile()
    return nc


def _fast_in_maps(x, wq, wk, wv, wo):
    import ml_dtypes

    E4 = ml_dtypes.float8_e4m3
    BF = ml_dtypes.bfloat16

    def split8(a, s):
        a_s = (np.asarray(a, np.float32) * np.float32(s))
        if np.abs(a_s).max() >= 224.0:
            return None
        hi = a_s.astype(E4)
        lo = (a_s - hi.astype(np.float32)).astype(E4)
        return hi, lo

    def chunked(a):  # [rows, cols] -> [128, rows//128, cols]
        return np.ascontiguousarray(
            a.reshape(a.shape[0] // 128, 128, -1).transpose(1, 0, 2))

    xs = [split8(x[b].T, SX) for b in range(B)]
    ws = {n: split8(w, SW) for n, w in (("q", wq), ("k", wk), ("v", wv))}
    if any(v is None for v in xs) or any(v is None for v in ws.values()):
        return None  # scale overflow -> fall back to slow path

    wo16_np = chunked(np.asarray(wo, np.float32).astype(BF))

    in_maps = []
    for c in range(NCORES):
        b, hg = c // 2, c % 2
        sl = slice(512 * hg, 512 * hg + 512)
        m = {
            "xh": chunked(xs[b][0]), "xl": chunked(xs[b][1]),
            "wqh": chunked(ws["q"][0][:, sl]), "wql": chunked(ws["q"][1][:, sl]),
            "wkh": chunked(ws["k"][0][:, sl]), "wkl": chunked(ws["k"][1][:, sl]),
            "wvh": chunked(ws["v"][0][:, sl]), "wvl": chunked(ws["v"][1][:, sl]),
            "wo16": wo16_np,
        }
        in_maps.append(m)
    return in_maps


def kernel(inputs, mask, wq, bq, wk, bk, wv, bv, wo, bo):
    from concourse.bass_utils import run_bass_kernel_spmd

    x = np.asarray(inputs, dtype=np.float32)
    wq = np.asarray(wq, dtype=np.float32)
    wk = np.asarray(wk, dtype=np.float32)
    wv = np.asarray(wv, dtype=np.float32)
    wo = np.asarray(wo, dtype=np.float32)
    bq = np.asarray(bq, dtype=np.float32)
    bk = np.asarray(bk, dtype=np.float32)
    bv = np.asarray(bv, dtype=np.float32)
    bo = np.asarray(bo, dtype=np.float32)
    mask2d = np.asarray(mask, dtype=np.float32).reshape(S, S)
    causal_ref = 1.0 - np.tril(np.ones((S, S), dtype=np.float32))
    causal = bool(np.array_equal(mask2d, causal_ref))
    zero_b = not (np.any(bq != 0) or np.any(bk != 0))

    in_maps = None
    if causal and zero_b:
        in_maps = _fast_in_maps(x, wq, wk, wv, wo)

    if in_maps is not None:
        if "causal" not in _CACHE:
            _CACHE["causal"] = _build_fast()
        nc = _CACHE["causal"]
    else:
        variant = "slow_causal" if causal else "slow_generic"
        if variant not in _CACHE:
            _CACHE[variant] = _build_slow(variant)
        nc = _CACHE[variant]
        in_maps = []
        for c in range(NCORES):
            b, hg = c // 2, c % 2
            sl = slice(512 * hg, 512 * hg + 512)
            m = {
                "xt": np.ascontiguousarray(
                    x[b].T.reshape(EC, 128, S).transpose(1, 0, 2)),
                "wq": np.ascontiguousarray(
                    wq[:, sl].reshape(EC, 128, 512).transpose(1, 0, 2)),
                "wk": np.ascontiguousarray(
                    wk[:, sl].reshape(EC, 128, 512).transpose(1, 0, 2)),
                "wv": np.ascontiguousarray(
                    wv[:, sl].reshape(EC, 128, 512).transpose(1, 0, 2)),
                "wo": np.ascontiguousarray(
                    wo.reshape(EC, 128, 1024).transpose(1, 0, 2)),
                "bq": np.ascontiguousarray(bq[sl].reshape(4, 128).T),
                "bk": np.ascontiguousarray(bk[sl].reshape(4, 128).T),
            }
            if not causal:
                m["mkt"] = np.ascontiguousarray(
                    (mask2d.T * np.float32(-2e9)).reshape(TT, 128, S)
                    .transpose(1, 0, 2))
            in_maps.append(m)

    res = run_bass_kernel_spmd(nc, in_maps, core_ids=list(range(NCORES)))
    full = np.empty((B, S, E), dtype=np.float32)
    for c in range(NCORES):
        b, hg = c // 2, c % 2
        o = np.asarray(res.results[c]["out"], dtype=np.float32)
        if o.shape == (4, 2, 128, 512):   # fast path layout
            o = o.transpose(0, 2, 1, 3).reshape(512, 1024)
        else:
            o = o.reshape(512, 1024)
        full[b, 512 * hg:512 * hg + 512, :] = o

    # biases bv/bo are zero in this problem; fold in exactly if ever nonzero.
    if np.any(bv != 0):
        bmat = np.zeros((S, E), dtype=np.float64)
        tpr = np.arange(S)
        e = np.arange(E)
        bmat[:, :] = bv[(64 * (tpr[:, None] // 64) + e[None, :] % 64)]
        full += (bmat @ np.asarray(wo, dtype=np.float64)).astype(np.float32)[None]
    if np.any(bo != 0):
        full += bo[None, None, :]
    return full
